# revision 5
# baseline (speedup 1.0000x reference)
"""Trainium2 Bass kernel for nn_CatConLayers (multi-head cross-attention over
time/category embeddings).

Sharding: 8 cores = 4 batches x 2 head-pairs. Each core computes, for its
batch b and heads {2g, 2g+1}:
    k_in^T build (time-embedding rows DMA'd, category embeddings gathered
    on-device via indirect DMA + PE transpose), hk/hq projections, scores^T,
    exp (softmax numerator; scores are tiny so no max-subtraction needed),
    value matmul, softmax denominator via ones-vector matmul, normalization,
    and the per-head output projection with Wo.
Host: shards inputs, sums the two head-pair partial outputs per batch, adds bo.

All device math is fp32. The KQ dimension is permuted (sin block | cos block |
emb0 | emb1) so the interleaved sin/cos layout of the reference never has to be
materialized on-chip; Wq/Wk rows and q_in^T are permuted identically on host.
"""

import numpy as np

import concourse.bass as bass
import concourse.mybir as mybir
import concourse.tile as tile
from concourse import bacc
from concourse.bass_utils import run_bass_kernel_spmd
from concourse.masks import make_identity

# Problem shapes (hardcoded per harness contract)
N, T, H, KQ, LD, NREF, DT = 4, 1024, 4, 128, 128, 128, 64
NCORES = 8
TCH = T // 128  # 8 key chunks of 128

F32 = mybir.dt.float32
I32 = mybir.dt.int32
AF = mybir.ActivationFunctionType

_CACHE = {}


def _build_program():
    nc = bacc.Bacc("TRN2", target_bir_lowering=False, debug=False,
                   num_devices=NCORES)

    kt_d = nc.dram_tensor("kt_time", [DT, T], F32, kind="ExternalInput")
    idx0_d = nc.dram_tensor("idx0", [TCH, 128], I32, kind="ExternalInput")
    idx1_d = nc.dram_tensor("idx1", [TCH, 128], I32, kind="ExternalInput")
    emb0_d = nc.dram_tensor("emb0", [101, 32], F32, kind="ExternalInput")
    emb1_d = nc.dram_tensor("emb1", [51, 32], F32, kind="ExternalInput")
    x_d = nc.dram_tensor("x", [T, LD], F32, kind="ExternalInput")
    qT_d = nc.dram_tensor("qT", [KQ, NREF], F32, kind="ExternalInput")
    wq_d = nc.dram_tensor("wq", [KQ, 2 * KQ], F32, kind="ExternalInput")
    wk_d = nc.dram_tensor("wk", [KQ, 2 * KQ], F32, kind="ExternalInput")
    wo_d = nc.dram_tensor("wo", [2 * LD, LD], F32, kind="ExternalInput")
    bq_d = nc.dram_tensor("bq2", [2, KQ], F32, kind="ExternalInput")
    bk_d = nc.dram_tensor("bk2", [2, KQ], F32, kind="ExternalInput")
    out_d = nc.dram_tensor("out", [NREF, LD], F32, kind="ExternalOutput")

    inv_sqrt_kq = float(1.0 / np.sqrt(KQ))

    with tile.TileContext(nc) as tc:
        with tc.tile_pool(name="const", bufs=1) as cp, \
             tc.tile_pool(name="work", bufs=2) as sp, \
             tc.tile_pool(name="ps", bufs=2, space="PSUM") as pp:

            ident = cp.tile([128, 128], F32)
            make_identity(nc, ident[:])
            ones_col = cp.tile([128, 1], F32)
            nc.gpsimd.memset(ones_col[:], 1.0)
            one11 = cp.tile([1, 1], F32)
            nc.gpsimd.memset(one11[:], 1.0)

            qT_sb = cp.tile([KQ, NREF], F32)
            nc.sync.dma_start(out=qT_sb[:], in_=qT_d[:])
            wq_sb = cp.tile([KQ, 2 * KQ], F32)
            nc.sync.dma_start(out=wq_sb[:], in_=wq_d[:])
            wk_sb = cp.tile([KQ, 2 * KQ], F32)
            nc.sync.dma_start(out=wk_sb[:], in_=wk_d[:])
            wo_sb = cp.tile([LD, 2 * LD], F32)  # head h at cols [h*LD, (h+1)*LD)
            for h in range(2):
                nc.sync.dma_start(out=wo_sb[:, h * LD:(h + 1) * LD],
                                  in_=wo_d[h * LD:(h + 1) * LD, :])
            bq_sb = cp.tile([KQ, 2], F32)
            bk_sb = cp.tile([KQ, 2], F32)
            for h in range(2):
                nc.sync.dma_start(out=bq_sb[:, h:h + 1], in_=bq_d[h, :, None])
                nc.sync.dma_start(out=bk_sb[:, h:h + 1], in_=bk_d[h, :, None])

            xall = cp.tile([128, T], F32)  # x chunk c at cols [c*128, (c+1)*128)
            for c in range(TCH):
                nc.sync.dma_start(out=xall[:, c * 128:(c + 1) * 128],
                                  in_=x_d[c * 128:(c + 1) * 128, :])

            # ---- k_in^T (permuted): rows 0:64 = time embedding, 64:128 = cats
            kT = cp.tile([KQ, T], F32)
            nc.sync.dma_start(out=kT[0:DT, :], in_=kt_d[:])
            for c in range(TCH):
                i0 = sp.tile([128, 1], I32, tag="i0")
                i1 = sp.tile([128, 1], I32, tag="i1")
                nc.sync.dma_start(out=i0[:], in_=idx0_d[c, :, None])
                nc.sync.dma_start(out=i1[:], in_=idx1_d[c, :, None])
                g = sp.tile([128, 64], F32, tag="g", bufs=3)
                nc.gpsimd.indirect_dma_start(
                    out=g[:, 0:32], out_offset=None, in_=emb0_d[:],
                    in_offset=bass.IndirectOffsetOnAxis(ap=i0[:, :1], axis=0))
                nc.gpsimd.indirect_dma_start(
                    out=g[:, 32:64], out_offset=None, in_=emb1_d[:],
                    in_offset=bass.IndirectOffsetOnAxis(ap=i1[:, :1], axis=0))
                zt = pp.tile([64, 128], F32, tag="s1", bufs=4)
                nc.tensor.transpose(out=zt[:], in_=g[:], identity=ident[:])
                nc.vector.tensor_copy(out=kT[DT:128, c * 128:(c + 1) * 128],
                                      in_=zt[:])

            # ---- hk^T / hq^T per local head
            hks, hqs = [], []
            for h in range(2):
                hp = pp.tile([128, T], F32, tag="w2", bufs=2)
                nc.tensor.matmul(out=hp[:, 0:512],
                                 lhsT=wk_sb[:, h * 128:(h + 1) * 128],
                                 rhs=kT[:, 0:512], start=True, stop=True)
                nc.tensor.matmul(out=hp[:, 512:1024],
                                 lhsT=wk_sb[:, h * 128:(h + 1) * 128],
                                 rhs=kT[:, 512:1024], start=True, stop=True)
                hs = sp.tile([128, T], F32, tag="hks", bufs=2)
                if h == 0:
                    nc.vector.tensor_scalar_add(out=hs[:], in0=hp[:],
                                                scalar1=bk_sb[:, 0:1])
                else:
                    nc.scalar.activation(out=hs[:], in_=hp[:], func=AF.Identity,
                                         bias=bk_sb[:, 1:2], scale=1.0)
                hks.append(hs)

                qp = pp.tile([128, NREF], F32, tag="s1", bufs=4)
                nc.tensor.matmul(out=qp[:],
                                 lhsT=wq_sb[:, h * 128:(h + 1) * 128],
                                 rhs=qT_sb[:], start=True, stop=True)
                qs = sp.tile([128, NREF], F32, tag="hqs", bufs=2)
                nc.vector.tensor_scalar_add(out=qs[:], in0=qp[:],
                                            scalar1=bq_sb[:, h:h + 1])
                hqs.append(qs)

            # ---- scores^T chunks + exp (softmax numerator, unnormalized)
            # p̃^T for head h, chunk c lives at pT_all[:, h*T + c*128 ...]
            pT_all = cp.tile([128, 2 * T], F32)
            for h in range(2):
                for cg in range(TCH // 4):
                    sc4 = pp.tile([128, 512], F32, tag="s1", bufs=4)
                    for j in range(4):
                        c = cg * 4 + j
                        nc.tensor.matmul(
                            out=sc4[:, j * 128:(j + 1) * 128],
                            lhsT=hks[h][:, c * 128:(c + 1) * 128],
                            rhs=hqs[h][:], start=True, stop=True)
                    nc.scalar.activation(
                        out=pT_all[:, h * T + cg * 512: h * T + (cg + 1) * 512],
                        in_=sc4[:], func=AF.Exp, scale=inv_sqrt_kq)

            # ---- value matmul: out_h^T[v, q] += x_c^T(stationary) @ p̃T_c
            vo = [pp.tile([128, NREF], F32, tag="w2", bufs=2, name=f"vo{h}")
                  for h in range(2)]
            for c in range(TCH):
                for h in range(2):
                    nc.tensor.matmul(
                        out=vo[h][:],
                        lhsT=xall[:, c * 128:(c + 1) * 128],
                        rhs=pT_all[:, h * T + c * 128: h * T + (c + 1) * 128],
                        start=(c == 0), stop=(c == TCH - 1))

            # ---- softmax denominators: Z_h[q] = ones^T @ p̃T (row), then
            # transpose to a column via a second tiny matmul, reciprocal.
            zrow = [pp.tile([1, NREF], F32, tag="s1", bufs=4, name=f"zr{h}")
                    for h in range(2)]
            for c in range(TCH):
                for h in range(2):
                    nc.tensor.matmul(
                        out=zrow[h][:], lhsT=ones_col[:],
                        rhs=pT_all[:, h * T + c * 128: h * T + (c + 1) * 128],
                        start=(c == 0), stop=(c == TCH - 1))
            rinv = []
            for h in range(2):
                zr_sb = sp.tile([1, NREF], F32, tag="zrs", bufs=2)
                nc.vector.tensor_copy(out=zr_sb[:], in_=zrow[h][:])
                zc_ps = pp.tile([NREF, 1], F32, tag="s1", bufs=4)
                nc.tensor.matmul(out=zc_ps[:], lhsT=zr_sb[:], rhs=one11[:],
                                 start=True, stop=True)
                zc_sb = sp.tile([NREF, 1], F32, tag="zcs", bufs=2)
                nc.vector.tensor_copy(out=zc_sb[:], in_=zc_ps[:])
                ri = sp.tile([NREF, 1], F32, tag="ri", bufs=2)
                nc.vector.reciprocal(out=ri[:], in_=zc_sb[:])
                rinv.append(ri)

            # ---- output projection per head, then normalize+combine
            fin = []
            for h in range(2):
                ot = sp.tile([128, NREF], F32, tag="ots", bufs=2)
                if h == 0:
                    nc.vector.tensor_copy(out=ot[:], in_=vo[h][:])
                else:
                    nc.scalar.copy(out=ot[:], in_=vo[h][:])
                fp = pp.tile([NREF, LD], F32, tag="s1", bufs=4, name=f"fin{h}")
                nc.tensor.matmul(out=fp[:], lhsT=ot[:],
                                 rhs=wo_sb[:, h * LD:(h + 1) * LD],
                                 start=True, stop=True)
                fin.append(fp)

            res0 = sp.tile([NREF, LD], F32, tag="res0", bufs=1)
            res1 = sp.tile([NREF, LD], F32, tag="res1", bufs=1)
            nc.vector.tensor_scalar_mul(out=res0[:], in0=fin[0][:],
                                        scalar1=rinv[0][:, :1])
            nc.vector.tensor_scalar_mul(out=res1[:], in0=fin[1][:],
                                        scalar1=rinv[1][:, :1])
            nc.vector.tensor_add(out=res0[:], in0=res0[:], in1=res1[:])
            nc.sync.dma_start(out=out_d[:], in_=res0[:])

    nc.compile()
    return nc


def _get_program():
    if "nc" not in _CACHE:
        _CACHE["nc"] = _build_program()
    return _CACHE["nc"]


def _host_prep(ts, emb0, emb1):
    """Time embeddings (permuted: sin block | cos block) and q_in^T."""
    div = np.exp(np.arange(0, DT, 2, dtype=np.float32)
                 * (-np.log(10.0) / DT)).astype(np.float32)  # (32,)
    ang = 48.0 * ts[:, :, None].astype(np.float32) * div[None, None, :]
    # kt_time[b]: (64, T): rows 0:32 sin, 32:64 cos
    kt = np.concatenate([np.sin(ang), np.cos(ang)],
                        axis=2).transpose(0, 2, 1)  # (N, 64, T)
    kt = np.ascontiguousarray(kt, dtype=np.float32)

    ref = np.linspace(0.0, 1.0, NREF, dtype=np.float32)
    ang_r = 48.0 * ref[:, None] * div[None, :]  # (NREF, 32)
    qT = np.empty((KQ, NREF), np.float32)
    qT[0:32] = np.sin(ang_r).T
    qT[32:64] = np.cos(ang_r).T
    qT[64:96] = np.asarray(emb0)[100][:, None]
    qT[96:128] = np.asarray(emb1)[50][:, None]
    return kt, qT


def kernel(ts, ys0, ys1, x, emb0, emb1, Wq, bq, Wk, bk, Wo, bo):
    ts = np.asarray(ts, np.float32)
    x = np.asarray(x, np.float32)
    emb0 = np.asarray(emb0, np.float32)
    emb1 = np.asarray(emb1, np.float32)
    Wq = np.asarray(Wq, np.float32)
    Wk = np.asarray(Wk, np.float32)
    Wo = np.asarray(Wo, np.float32)
    bq = np.asarray(bq, np.float32)
    bk = np.asarray(bk, np.float32)
    bo = np.asarray(bo, np.float32)
    i0 = np.ascontiguousarray(np.asarray(ys0).astype(np.int32).reshape(N, TCH, 128))
    i1 = np.ascontiguousarray(np.asarray(ys1).astype(np.int32).reshape(N, TCH, 128))

    kt, qT = _host_prep(ts, emb0, emb1)
    # KQ permutation: (sin block | cos block | emb0 | emb1) -> reference order
    perm = np.concatenate([2 * np.arange(32), 2 * np.arange(32) + 1,
                           64 + np.arange(32), 96 + np.arange(32)])
    Wq_p = np.ascontiguousarray(Wq[perm])
    Wk_p = np.ascontiguousarray(Wk[perm])
    bq2 = bq.reshape(H, KQ)
    bk2 = bk.reshape(H, KQ)

    in_maps = []
    for c in range(NCORES):
        b, hg = c // 2, c % 2
        in_maps.append(dict(
            kt_time=kt[b],
            idx0=i0[b], idx1=i1[b],
            emb0=emb0, emb1=emb1,
            x=np.ascontiguousarray(x[b]),
            qT=qT,
            wq=np.ascontiguousarray(Wq_p[:, hg * 256:(hg + 1) * 256]),
            wk=np.ascontiguousarray(Wk_p[:, hg * 256:(hg + 1) * 256]),
            wo=np.ascontiguousarray(Wo[hg * 256:(hg + 1) * 256, :]),
            bq2=np.ascontiguousarray(bq2[2 * hg:2 * hg + 2]),
            bk2=np.ascontiguousarray(bk2[2 * hg:2 * hg + 2]),
        ))

    nc = _get_program()
    res = run_bass_kernel_spmd(nc, in_maps, list(range(NCORES)))
    out = np.empty((N, NREF, LD), np.float32)
    for b in range(N):
        out[b] = (res.results[2 * b]["out"] + res.results[2 * b + 1]["out"]
                  + bo[None, :])
    return out


# revision 7
# speedup vs baseline: 2.3742x; 2.3742x over previous
"""Trainium2 Bass kernel for nn_CatConLayers (multi-head cross-attention over
time/category embeddings).

Sharding: 8 cores = 4 batches x 2 head-pairs. Each core computes, for its
batch b and heads {2g, 2g+1}: hk/hq projections of k_in^T / q_in^T,
scores^T = hk_h^T-chunks @ hq_h, exp (softmax numerator; scores are tiny so
no max-subtraction is needed), the value matmul accumulated over key chunks,
the softmax denominator via a ones-vector matmul, normalization, and the
per-head output projection with Wo. Host: builds k_in^T/q_in^T featurization
(sinusoidal time embedding + category-embedding rows; the ACT Sin table
cannot be co-resident with the Exp table, and on-device indirect-DMA gathers
measured 1.1us each), shards inputs, sums the two head-pair partials per
batch, adds bo.

Matmul operands on the scores path are bf16 (fp32 PSUM accumulation); the
value/output path dtype is selectable (fp32 default for accuracy).

The KQ dimension is permuted (sin block | cos block | emb0 | emb1) so the
interleaved sin/cos layout of the reference never has to be materialized
on-chip; Wq/Wk rows and q_in^T are permuted identically on host.
"""

import numpy as np
import ml_dtypes

import concourse.bass as bass
import concourse.mybir as mybir
import concourse.tile as tile
from concourse import bacc
from concourse.bass_utils import run_bass_kernel_spmd

# Problem shapes (hardcoded per harness contract)
N, T, H, KQ, LD, NREF, DT = 4, 1024, 4, 128, 128, 128, 64
NCORES = 8
TCH = T // 128  # 8 key chunks of 128

F32 = mybir.dt.float32
BF16 = mybir.dt.bfloat16
AF = mybir.ActivationFunctionType

VALUE_DTYPE = "f32"  # "f32" (accurate) or "bf16" (faster)

_CACHE = {}


def _build_program(vd_name):
    VD = F32 if vd_name == "f32" else BF16
    nc = bacc.Bacc("TRN2", target_bir_lowering=False, debug=False,
                   num_devices=NCORES)

    kT_d = nc.dram_tensor("kT", [KQ, T], BF16, kind="ExternalInput")
    x_d = nc.dram_tensor("xr", [128, T], VD, kind="ExternalInput")
    qT_d = nc.dram_tensor("qT", [KQ, NREF], BF16, kind="ExternalInput")
    wq_d = nc.dram_tensor("wq", [KQ, 2 * KQ], BF16, kind="ExternalInput")
    wk_d = nc.dram_tensor("wk", [KQ, 2 * KQ], BF16, kind="ExternalInput")
    wo_d = nc.dram_tensor("wo", [LD, 2 * LD], VD, kind="ExternalInput")
    bq_d = nc.dram_tensor("bq2", [2, KQ], F32, kind="ExternalInput")
    bk_d = nc.dram_tensor("bk2", [2, KQ], F32, kind="ExternalInput")
    out_d = nc.dram_tensor("out", [NREF, LD], F32, kind="ExternalOutput")

    inv_sqrt_kq = float(1.0 / np.sqrt(KQ))

    with tile.TileContext(nc) as tc:
        with tc.tile_pool(name="const", bufs=1) as cp, \
             tc.tile_pool(name="work", bufs=2) as sp, \
             tc.tile_pool(name="ps", bufs=2, space="PSUM") as pp:

            ones_col = cp.tile([128, 1], VD)
            nc.gpsimd.memset(ones_col[:], 1.0)
            one11 = cp.tile([1, 1], F32)
            nc.gpsimd.memset(one11[:], 1.0)

            kT = cp.tile([KQ, T], BF16)
            nc.sync.dma_start(out=kT[:], in_=kT_d[:])
            xall = cp.tile([128, T], VD)  # x chunk c at cols [c*128,(c+1)*128)
            nc.gpsimd.dma_start(out=xall[:], in_=x_d[:])
            qT_sb = cp.tile([KQ, NREF], BF16)
            nc.scalar.dma_start(out=qT_sb[:], in_=qT_d[:])
            wq_sb = cp.tile([KQ, 2 * KQ], BF16)
            nc.scalar.dma_start(out=wq_sb[:], in_=wq_d[:])
            wk_sb = cp.tile([KQ, 2 * KQ], BF16)
            nc.sync.dma_start(out=wk_sb[:], in_=wk_d[:])
            wo_sb = cp.tile([LD, 2 * LD], VD)  # head h at cols [h*LD,(h+1)*LD)
            nc.gpsimd.dma_start(out=wo_sb[:], in_=wo_d[:])
            bq_sb = cp.tile([KQ, 2], F32)
            bk_sb = cp.tile([KQ, 2], F32)
            for h in range(2):
                nc.scalar.dma_start(out=bq_sb[:, h:h + 1], in_=bq_d[h, :, None])
                nc.sync.dma_start(out=bk_sb[:, h:h + 1], in_=bk_d[h, :, None])

            # ---- hk^T / hq^T per local head (bf16 out, fp32 bias add)
            hks, hqs = [], []
            for h in range(2):
                hp = pp.tile([128, T], F32, tag="w2", bufs=2)
                nc.tensor.matmul(out=hp[:, 0:512],
                                 lhsT=wk_sb[:, h * 128:(h + 1) * 128],
                                 rhs=kT[:, 0:512], start=True, stop=True)
                nc.tensor.matmul(out=hp[:, 512:1024],
                                 lhsT=wk_sb[:, h * 128:(h + 1) * 128],
                                 rhs=kT[:, 512:1024], start=True, stop=True)
                hs = sp.tile([128, T], BF16, tag="hks", bufs=2)
                if h == 0:
                    nc.vector.tensor_scalar_add(out=hs[:], in0=hp[:],
                                                scalar1=bk_sb[:, 0:1])
                else:
                    nc.scalar.activation(out=hs[:], in_=hp[:], func=AF.Identity,
                                         bias=bk_sb[:, 1:2], scale=1.0)
                hks.append(hs)

                qp = pp.tile([128, NREF], F32, tag="s1", bufs=4)
                nc.tensor.matmul(out=qp[:],
                                 lhsT=wq_sb[:, h * 128:(h + 1) * 128],
                                 rhs=qT_sb[:], start=True, stop=True)
                qs = sp.tile([128, NREF], BF16, tag="hqs", bufs=2)
                nc.vector.tensor_scalar_add(out=qs[:], in0=qp[:],
                                            scalar1=bq_sb[:, h:h + 1])
                hqs.append(qs)

            # ---- scores^T chunks + exp (softmax numerator, unnormalized)
            # p~^T for head h, chunk c lives at pT_all[:, h*T + c*128 ...]
            pT_all = cp.tile([128, 2 * T], VD)
            for h in range(2):
                for cg in range(TCH // 4):
                    sc4 = pp.tile([128, 512], F32, tag="s1", bufs=4)
                    for j in range(4):
                        c = cg * 4 + j
                        nc.tensor.matmul(
                            out=sc4[:, j * 128:(j + 1) * 128],
                            lhsT=hks[h][:, c * 128:(c + 1) * 128],
                            rhs=hqs[h][:], start=True, stop=True)
                    nc.scalar.activation(
                        out=pT_all[:, h * T + cg * 512: h * T + (cg + 1) * 512],
                        in_=sc4[:], func=AF.Exp, scale=inv_sqrt_kq)

            # ---- value matmul: out_h^T[v, q] += x_c^T(stationary) @ p~T_c
            vo = [pp.tile([128, NREF], F32, tag="w2", bufs=2, name=f"vo{h}")
                  for h in range(2)]
            for c in range(TCH):
                for h in range(2):
                    nc.tensor.matmul(
                        out=vo[h][:],
                        lhsT=xall[:, c * 128:(c + 1) * 128],
                        rhs=pT_all[:, h * T + c * 128: h * T + (c + 1) * 128],
                        start=(c == 0), stop=(c == TCH - 1))

            # ---- softmax denominators: Z_h[q] as a row via ones^T @ p~T,
            # transposed to a column by a second tiny matmul, then 1/Z.
            zrow = [pp.tile([1, NREF], F32, tag="s1", bufs=4, name=f"zr{h}")
                    for h in range(2)]
            for c in range(TCH):
                for h in range(2):
                    nc.tensor.matmul(
                        out=zrow[h][:], lhsT=ones_col[:],
                        rhs=pT_all[:, h * T + c * 128: h * T + (c + 1) * 128],
                        start=(c == 0), stop=(c == TCH - 1))
            rinv = []
            for h in range(2):
                zr_sb = sp.tile([1, NREF], F32, tag="zrs", bufs=2)
                nc.vector.tensor_copy(out=zr_sb[:], in_=zrow[h][:])
                zc_ps = pp.tile([NREF, 1], F32, tag="s1", bufs=4)
                nc.tensor.matmul(out=zc_ps[:], lhsT=zr_sb[:], rhs=one11[:],
                                 start=True, stop=True)
                zc_sb = sp.tile([NREF, 1], F32, tag="zcs", bufs=2)
                nc.vector.tensor_copy(out=zc_sb[:], in_=zc_ps[:])
                ri = sp.tile([NREF, 1], F32, tag="ri", bufs=2)
                nc.vector.reciprocal(out=ri[:], in_=zc_sb[:])
                rinv.append(ri)

            # ---- output projection per head, then normalize+combine
            fin = []
            for h in range(2):
                ot = sp.tile([128, NREF], VD, tag="ots", bufs=2)
                if h == 0:
                    nc.vector.tensor_copy(out=ot[:], in_=vo[h][:])
                else:
                    nc.scalar.copy(out=ot[:], in_=vo[h][:])
                fp = pp.tile([NREF, LD], F32, tag="s1", bufs=4, name=f"fin{h}")
                nc.tensor.matmul(out=fp[:], lhsT=ot[:],
                                 rhs=wo_sb[:, h * LD:(h + 1) * LD],
                                 start=True, stop=True)
                fin.append(fp)

            res0 = sp.tile([NREF, LD], F32, tag="res0", bufs=1)
            res1 = sp.tile([NREF, LD], F32, tag="res1", bufs=1)
            nc.vector.tensor_scalar_mul(out=res0[:], in0=fin[0][:],
                                        scalar1=rinv[0][:, :1])
            nc.vector.tensor_scalar_mul(out=res1[:], in0=fin[1][:],
                                        scalar1=rinv[1][:, :1])
            nc.vector.tensor_add(out=res0[:], in0=res0[:], in1=res1[:])
            nc.sync.dma_start(out=out_d[:], in_=res0[:])

    nc.compile()
    return nc


def _get_program(vd_name=None):
    vd_name = vd_name or VALUE_DTYPE
    if vd_name not in _CACHE:
        _CACHE[vd_name] = _build_program(vd_name)
    return _CACHE[vd_name]


def _host_prep(ts, ys0, ys1, emb0, emb1):
    """Full k_in^T (permuted) per batch and q_in^T."""
    div = np.exp(np.arange(0, DT, 2, dtype=np.float32)
                 * (-np.log(10.0) / DT)).astype(np.float32)  # (32,)
    ang = 48.0 * ts[:, :, None].astype(np.float32) * div[None, None, :]
    kT = np.empty((N, KQ, T), np.float32)
    kT[:, 0:32] = np.sin(ang).transpose(0, 2, 1)
    kT[:, 32:64] = np.cos(ang).transpose(0, 2, 1)
    kT[:, 64:96] = emb0[ys0].transpose(0, 2, 1)
    kT[:, 96:128] = emb1[ys1].transpose(0, 2, 1)

    ref = np.linspace(0.0, 1.0, NREF, dtype=np.float32)
    ang_r = 48.0 * ref[:, None] * div[None, :]  # (NREF, 32)
    qT = np.empty((KQ, NREF), np.float32)
    qT[0:32] = np.sin(ang_r).T
    qT[32:64] = np.cos(ang_r).T
    qT[64:96] = emb0[100][:, None]
    qT[96:128] = emb1[50][:, None]
    return kT, qT


def _make_in_maps(ts, ys0, ys1, x, emb0, emb1, Wq, bq, Wk, bk, Wo, vd_name):
    vd = np.float32 if vd_name == "f32" else ml_dtypes.bfloat16
    bf = ml_dtypes.bfloat16
    ts = np.asarray(ts, np.float32)
    x = np.asarray(x, np.float32)
    emb0 = np.asarray(emb0, np.float32)
    emb1 = np.asarray(emb1, np.float32)
    ys0 = np.asarray(ys0).astype(np.int64)
    ys1 = np.asarray(ys1).astype(np.int64)

    kT, qT = _host_prep(ts, ys0, ys1, emb0, emb1)
    # KQ permutation: (sin block | cos block | emb0 | emb1) -> reference order
    perm = np.concatenate([2 * np.arange(32), 2 * np.arange(32) + 1,
                           64 + np.arange(32), 96 + np.arange(32)])
    Wq_p = np.asarray(Wq, np.float32)[perm]
    Wk_p = np.asarray(Wk, np.float32)[perm]
    bq2 = np.asarray(bq, np.float32).reshape(H, KQ)
    bk2 = np.asarray(bk, np.float32).reshape(H, KQ)
    Wo = np.asarray(Wo, np.float32)
    # x rearranged: chunk c on cols [c*128,(c+1)*128), key t=c*128+p on part p
    xr = np.ascontiguousarray(
        x.reshape(N, TCH, 128, LD).transpose(0, 2, 1, 3).reshape(N, 128, T))

    in_maps = []
    for c in range(NCORES):
        b, hg = c // 2, c % 2
        # wo laid out (LD, 2*LD): local head h rows at cols [h*LD,(h+1)*LD)
        wo2 = np.ascontiguousarray(
            Wo[hg * 256:(hg + 1) * 256, :].reshape(2, LD, LD)
            .transpose(1, 0, 2).reshape(LD, 2 * LD))
        in_maps.append(dict(
            kT=kT[b].astype(bf),
            xr=xr[b].astype(vd),
            qT=qT.astype(bf),
            wq=np.ascontiguousarray(Wq_p[:, hg * 256:(hg + 1) * 256]).astype(bf),
            wk=np.ascontiguousarray(Wk_p[:, hg * 256:(hg + 1) * 256]).astype(bf),
            wo=wo2.astype(vd),
            bq2=np.ascontiguousarray(bq2[2 * hg:2 * hg + 2]),
            bk2=np.ascontiguousarray(bk2[2 * hg:2 * hg + 2]),
        ))
    return in_maps


def kernel(ts, ys0, ys1, x, emb0, emb1, Wq, bq, Wk, bk, Wo, bo):
    in_maps = _make_in_maps(ts, ys0, ys1, x, emb0, emb1, Wq, bq, Wk, bk, Wo,
                            VALUE_DTYPE)
    nc = _get_program()
    res = run_bass_kernel_spmd(nc, in_maps, list(range(NCORES)))
    bo = np.asarray(bo, np.float32)
    out = np.empty((N, NREF, LD), np.float32)
    for b in range(N):
        out[b] = (res.results[2 * b]["out"] + res.results[2 * b + 1]["out"]
                  + bo[None, :])
    return out


# revision 8
# speedup vs baseline: 2.8052x; 1.1815x over previous
"""Trainium2 Bass kernel for nn_CatConLayers (multi-head cross-attention over
time/category embeddings).

Sharding: 8 cores = 4 batches x 2 head-pairs. Each core computes, for its
batch b and heads {2g, 2g+1}: hk/hq projections of k_in^T / q_in^T,
scores^T = hk_h^T-chunks @ hq_h, exp (softmax numerator; scores are tiny so
no max-subtraction is needed), the value matmul accumulated over key chunks,
the softmax denominator via a ones-vector matmul, normalization, and the
per-head output projection with Wo. Host: builds k_in^T/q_in^T featurization
(sinusoidal time embedding + category-embedding rows; the ACT Sin table
cannot be co-resident with the Exp table, and on-device indirect-DMA gathers
measured 1.1us each), shards inputs, sums the two head-pair partials per
batch, adds bo.

Matmul operands on the scores path are bf16 (fp32 PSUM accumulation); the
value/output path dtype is selectable (fp32 default for accuracy).

The KQ dimension is permuted (sin block | cos block | emb0 | emb1) so the
interleaved sin/cos layout of the reference never has to be materialized
on-chip; Wq/Wk rows and q_in^T are permuted identically on host.
"""

import numpy as np
import ml_dtypes

import concourse.bass as bass
import concourse.mybir as mybir
import concourse.tile as tile
from concourse import bacc
from concourse.bass_utils import run_bass_kernel_spmd

# Problem shapes (hardcoded per harness contract)
N, T, H, KQ, LD, NREF, DT = 4, 1024, 4, 128, 128, 128, 64
NCORES = 8
TCH = T // 128  # 8 key chunks of 128

F32 = mybir.dt.float32
BF16 = mybir.dt.bfloat16
FP16 = mybir.dt.float16
AF = mybir.ActivationFunctionType

# matmul operand dtype scheme: "f16" = fp16 everywhere (1-pass matmuls,
# ~5e-4 absmax-rel), "hybrid" = bf16 scores + fp32 value (~6e-5, slower)
VALUE_DTYPE = "f16"

_CACHE = {}


def _build_program(vd_name):
    if vd_name == "f16":
        SD = VD = FP16
    elif vd_name == "bf16":
        SD = VD = BF16
    else:  # hybrid
        SD, VD = BF16, F32
    nc = bacc.Bacc("TRN2", target_bir_lowering=False, debug=False,
                   num_devices=NCORES)

    kT_d = nc.dram_tensor("kT", [KQ, T], SD, kind="ExternalInput")
    x_d = nc.dram_tensor("xr", [128, T], VD, kind="ExternalInput")
    qT_d = nc.dram_tensor("qT", [KQ, NREF], SD, kind="ExternalInput")
    wq_d = nc.dram_tensor("wq", [KQ, 2 * KQ], SD, kind="ExternalInput")
    wk_d = nc.dram_tensor("wk", [KQ, 2 * KQ], SD, kind="ExternalInput")
    wo_d = nc.dram_tensor("wo", [LD, 2 * LD], VD, kind="ExternalInput")
    bq_d = nc.dram_tensor("bq2", [2, KQ], F32, kind="ExternalInput")
    bk_d = nc.dram_tensor("bk2", [2, KQ], F32, kind="ExternalInput")
    out_d = nc.dram_tensor("out", [NREF, LD], F32, kind="ExternalOutput")

    inv_sqrt_kq = float(1.0 / np.sqrt(KQ))

    with tile.TileContext(nc) as tc:
        with tc.tile_pool(name="const", bufs=1) as cp, \
             tc.tile_pool(name="work", bufs=2) as sp, \
             tc.tile_pool(name="ps", bufs=2, space="PSUM") as pp:

            ones_col = cp.tile([128, 1], VD)
            nc.vector.memset(ones_col[:], 1.0)
            one11 = cp.tile([1, 1], F32)
            nc.vector.memset(one11[:], 1.0)

            kT = cp.tile([KQ, T], SD)
            nc.sync.dma_start(out=kT[:], in_=kT_d[:])
            xall = cp.tile([128, T], VD)  # x chunk c at cols [c*128,(c+1)*128)
            nc.sync.dma_start(out=xall[:], in_=x_d[:])
            qT_sb = cp.tile([KQ, NREF], SD)
            nc.scalar.dma_start(out=qT_sb[:], in_=qT_d[:])
            wq_sb = cp.tile([KQ, 2 * KQ], SD)
            nc.scalar.dma_start(out=wq_sb[:], in_=wq_d[:])
            wk_sb = cp.tile([KQ, 2 * KQ], SD)
            nc.sync.dma_start(out=wk_sb[:], in_=wk_d[:])
            wo_sb = cp.tile([LD, 2 * LD], VD)  # head h at cols [h*LD,(h+1)*LD)
            nc.scalar.dma_start(out=wo_sb[:], in_=wo_d[:])
            bq_sb = cp.tile([KQ, 2], F32)
            bk_sb = cp.tile([KQ, 2], F32)
            for h in range(2):
                nc.scalar.dma_start(out=bq_sb[:, h:h + 1], in_=bq_d[h, :, None])
                nc.sync.dma_start(out=bk_sb[:, h:h + 1], in_=bk_d[h, :, None])

            # ---- hk^T / hq^T per local head (bf16 out, fp32 bias add)
            hks, hqs = [], []
            for h in range(2):
                hp = pp.tile([128, T], F32, tag="w2", bufs=2)
                nc.tensor.matmul(out=hp[:, 0:512],
                                 lhsT=wk_sb[:, h * 128:(h + 1) * 128],
                                 rhs=kT[:, 0:512], start=True, stop=True)
                nc.tensor.matmul(out=hp[:, 512:1024],
                                 lhsT=wk_sb[:, h * 128:(h + 1) * 128],
                                 rhs=kT[:, 512:1024], start=True, stop=True)
                hs = sp.tile([128, T], SD, tag="hks", bufs=2)
                if h == 0:
                    nc.vector.tensor_scalar_add(out=hs[:], in0=hp[:],
                                                scalar1=bk_sb[:, 0:1])
                else:
                    nc.scalar.activation(out=hs[:], in_=hp[:], func=AF.Identity,
                                         bias=bk_sb[:, 1:2], scale=1.0)
                hks.append(hs)

                qp = pp.tile([128, NREF], F32, tag="s1", bufs=4)
                nc.tensor.matmul(out=qp[:],
                                 lhsT=wq_sb[:, h * 128:(h + 1) * 128],
                                 rhs=qT_sb[:], start=True, stop=True)
                qs = sp.tile([128, NREF], SD, tag="hqs", bufs=2)
                nc.vector.tensor_scalar_add(out=qs[:], in0=qp[:],
                                            scalar1=bq_sb[:, h:h + 1])
                hqs.append(qs)

            # ---- scores^T chunks + exp (softmax numerator, unnormalized)
            # p~^T for head h, chunk c lives at pT_all[:, h*T + c*128 ...]
            pT_all = cp.tile([128, 2 * T], VD)
            for h in range(2):
                for cg in range(TCH // 4):
                    sc4 = pp.tile([128, 512], F32, tag="s1", bufs=4)
                    for j in range(4):
                        c = cg * 4 + j
                        nc.tensor.matmul(
                            out=sc4[:, j * 128:(j + 1) * 128],
                            lhsT=hks[h][:, c * 128:(c + 1) * 128],
                            rhs=hqs[h][:], start=True, stop=True)
                    nc.scalar.activation(
                        out=pT_all[:, h * T + cg * 512: h * T + (cg + 1) * 512],
                        in_=sc4[:], func=AF.Exp, scale=inv_sqrt_kq)

            # ---- softmax denominators: Z_h[q] as a row via ones^T @ p~T,
            # transposed to a column by a second tiny matmul, then 1/Z.
            zrow = [pp.tile([1, NREF], F32, tag="s1", bufs=4, name=f"zr{h}")
                    for h in range(2)]
            for c in range(TCH):
                for h in range(2):
                    nc.tensor.matmul(
                        out=zrow[h][:], lhsT=ones_col[:],
                        rhs=pT_all[:, h * T + c * 128: h * T + (c + 1) * 128],
                        start=(c == 0), stop=(c == TCH - 1))

            # ---- value matmul: out_h^T[v, q] += x_c^T(stationary) @ p~T_c
            vo = [pp.tile([128, NREF], F32, tag="w2", bufs=2, name=f"vo{h}")
                  for h in range(2)]
            for c in range(TCH):
                for h in range(2):
                    nc.tensor.matmul(
                        out=vo[h][:],
                        lhsT=xall[:, c * 128:(c + 1) * 128],
                        rhs=pT_all[:, h * T + c * 128: h * T + (c + 1) * 128],
                        start=(c == 0), stop=(c == TCH - 1))

            rinv = []
            for h in range(2):
                zr_sb = sp.tile([1, NREF], F32, tag="zrs", bufs=2)
                nc.vector.tensor_copy(out=zr_sb[:], in_=zrow[h][:])
                zc_ps = pp.tile([NREF, 1], F32, tag="s1", bufs=4)
                nc.tensor.matmul(out=zc_ps[:], lhsT=zr_sb[:], rhs=one11[:],
                                 start=True, stop=True)
                zc_sb = sp.tile([NREF, 1], F32, tag="zcs", bufs=2)
                nc.vector.tensor_copy(out=zc_sb[:], in_=zc_ps[:])
                ri = sp.tile([NREF, 1], F32, tag="ri", bufs=2)
                nc.vector.reciprocal(out=ri[:], in_=zc_sb[:])
                rinv.append(ri)

            # ---- output projection per head, then normalize+combine
            fin = []
            for h in range(2):
                ot = sp.tile([128, NREF], VD, tag="ots", bufs=2)
                if h == 0:
                    nc.vector.tensor_copy(out=ot[:], in_=vo[h][:])
                else:
                    nc.scalar.copy(out=ot[:], in_=vo[h][:])
                fp = pp.tile([NREF, LD], F32, tag="s1", bufs=4, name=f"fin{h}")
                nc.tensor.matmul(out=fp[:], lhsT=ot[:],
                                 rhs=wo_sb[:, h * LD:(h + 1) * LD],
                                 start=True, stop=True)
                fin.append(fp)

            res0 = sp.tile([NREF, LD], F32, tag="res0", bufs=1)
            res1 = sp.tile([NREF, LD], F32, tag="res1", bufs=1)
            nc.vector.tensor_scalar_mul(out=res0[:], in0=fin[0][:],
                                        scalar1=rinv[0][:, :1])
            nc.vector.tensor_scalar_mul(out=res1[:], in0=fin[1][:],
                                        scalar1=rinv[1][:, :1])
            nc.vector.tensor_add(out=res0[:], in0=res0[:], in1=res1[:])
            nc.sync.dma_start(out=out_d[:], in_=res0[:])

    nc.compile()
    return nc


def _get_program(vd_name=None):
    vd_name = vd_name or VALUE_DTYPE
    if vd_name not in _CACHE:
        _CACHE[vd_name] = _build_program(vd_name)
    return _CACHE[vd_name]


def _host_prep(ts, ys0, ys1, emb0, emb1):
    """Full k_in^T (permuted) per batch and q_in^T."""
    div = np.exp(np.arange(0, DT, 2, dtype=np.float32)
                 * (-np.log(10.0) / DT)).astype(np.float32)  # (32,)
    ang = 48.0 * ts[:, :, None].astype(np.float32) * div[None, None, :]
    kT = np.empty((N, KQ, T), np.float32)
    kT[:, 0:32] = np.sin(ang).transpose(0, 2, 1)
    kT[:, 32:64] = np.cos(ang).transpose(0, 2, 1)
    kT[:, 64:96] = emb0[ys0].transpose(0, 2, 1)
    kT[:, 96:128] = emb1[ys1].transpose(0, 2, 1)

    ref = np.linspace(0.0, 1.0, NREF, dtype=np.float32)
    ang_r = 48.0 * ref[:, None] * div[None, :]  # (NREF, 32)
    qT = np.empty((KQ, NREF), np.float32)
    qT[0:32] = np.sin(ang_r).T
    qT[32:64] = np.cos(ang_r).T
    qT[64:96] = emb0[100][:, None]
    qT[96:128] = emb1[50][:, None]
    return kT, qT


def _make_in_maps(ts, ys0, ys1, x, emb0, emb1, Wq, bq, Wk, bk, Wo, vd_name):
    if vd_name == "f16":
        sd = vd = np.float16
    elif vd_name == "bf16":
        sd = vd = ml_dtypes.bfloat16
    else:  # hybrid
        sd, vd = ml_dtypes.bfloat16, np.float32
    bf = sd
    ts = np.asarray(ts, np.float32)
    x = np.asarray(x, np.float32)
    emb0 = np.asarray(emb0, np.float32)
    emb1 = np.asarray(emb1, np.float32)
    ys0 = np.asarray(ys0).astype(np.int64)
    ys1 = np.asarray(ys1).astype(np.int64)

    kT, qT = _host_prep(ts, ys0, ys1, emb0, emb1)
    # KQ permutation: (sin block | cos block | emb0 | emb1) -> reference order
    perm = np.concatenate([2 * np.arange(32), 2 * np.arange(32) + 1,
                           64 + np.arange(32), 96 + np.arange(32)])
    Wq_p = np.asarray(Wq, np.float32)[perm]
    Wk_p = np.asarray(Wk, np.float32)[perm]
    bq2 = np.asarray(bq, np.float32).reshape(H, KQ)
    bk2 = np.asarray(bk, np.float32).reshape(H, KQ)
    Wo = np.asarray(Wo, np.float32)
    # x rearranged: chunk c on cols [c*128,(c+1)*128), key t=c*128+p on part p
    xr = np.ascontiguousarray(
        x.reshape(N, TCH, 128, LD).transpose(0, 2, 1, 3).reshape(N, 128, T))

    in_maps = []
    for c in range(NCORES):
        b, hg = c // 2, c % 2
        # wo laid out (LD, 2*LD): local head h rows at cols [h*LD,(h+1)*LD)
        wo2 = np.ascontiguousarray(
            Wo[hg * 256:(hg + 1) * 256, :].reshape(2, LD, LD)
            .transpose(1, 0, 2).reshape(LD, 2 * LD))
        in_maps.append(dict(
            kT=kT[b].astype(bf),
            xr=xr[b].astype(vd),
            qT=qT.astype(bf),
            wq=np.ascontiguousarray(Wq_p[:, hg * 256:(hg + 1) * 256]).astype(bf),
            wk=np.ascontiguousarray(Wk_p[:, hg * 256:(hg + 1) * 256]).astype(bf),
            wo=wo2.astype(vd),
            bq2=np.ascontiguousarray(bq2[2 * hg:2 * hg + 2]),
            bk2=np.ascontiguousarray(bk2[2 * hg:2 * hg + 2]),
        ))
    return in_maps


def kernel(ts, ys0, ys1, x, emb0, emb1, Wq, bq, Wk, bk, Wo, bo):
    in_maps = _make_in_maps(ts, ys0, ys1, x, emb0, emb1, Wq, bq, Wk, bk, Wo,
                            VALUE_DTYPE)
    nc = _get_program()
    res = run_bass_kernel_spmd(nc, in_maps, list(range(NCORES)))
    bo = np.asarray(bo, np.float32)
    out = np.empty((N, NREF, LD), np.float32)
    for b in range(N):
        out[b] = (res.results[2 * b]["out"] + res.results[2 * b + 1]["out"]
                  + bo[None, :])
    return out


# revision 9
# speedup vs baseline: 2.8343x; 1.0104x over previous
"""Trainium2 Bass kernel for nn_CatConLayers (multi-head cross-attention over
time/category embeddings).

Sharding: 8 cores = 4 batches x 2 head-pairs. Each core computes, for its
batch b and heads {2g, 2g+1}: hk/hq projections of k_in^T / q_in^T,
scores^T = hk_h^T-chunks @ hq_h, exp (softmax numerator; scores are tiny so
no max-subtraction is needed), the value matmul accumulated over key chunks,
the softmax denominator via a ones-vector matmul, normalization, and the
per-head output projection with Wo. Host: builds k_in^T/q_in^T featurization
(sinusoidal time embedding + category-embedding rows; the ACT Sin table
cannot be co-resident with the Exp table, and on-device indirect-DMA gathers
measured 1.1us each), shards inputs, sums the two head-pair partials per
batch, adds bo.

Matmul operands on the scores path are bf16 (fp32 PSUM accumulation); the
value/output path dtype is selectable (fp32 default for accuracy).

The KQ dimension is permuted (sin block | cos block | emb0 | emb1) so the
interleaved sin/cos layout of the reference never has to be materialized
on-chip; Wq/Wk rows and q_in^T are permuted identically on host.
"""

import numpy as np
import ml_dtypes

import concourse.bass as bass
import concourse.mybir as mybir
import concourse.tile as tile
from concourse import bacc
from concourse.bass_utils import run_bass_kernel_spmd

# Problem shapes (hardcoded per harness contract)
N, T, H, KQ, LD, NREF, DT = 4, 1024, 4, 128, 128, 128, 64
NCORES = 8
TCH = T // 128  # 8 key chunks of 128

F32 = mybir.dt.float32
BF16 = mybir.dt.bfloat16
FP16 = mybir.dt.float16
AF = mybir.ActivationFunctionType

# matmul operand dtype scheme: "f16" = fp16 everywhere (1-pass matmuls,
# ~5e-4 absmax-rel), "hybrid" = bf16 scores + fp32 value (~6e-5, slower)
VALUE_DTYPE = "f16"

_CACHE = {}


def _build_program(vd_name):
    if vd_name == "f16":
        SD = VD = FP16
    elif vd_name == "bf16":
        SD = VD = BF16
    else:  # hybrid
        SD, VD = BF16, F32
    nc = bacc.Bacc("TRN2", target_bir_lowering=False, debug=False,
                   num_devices=NCORES)

    kT_d = nc.dram_tensor("kT", [KQ, T], SD, kind="ExternalInput")
    x_d = nc.dram_tensor("xr", [128, T], VD, kind="ExternalInput")
    qT_d = nc.dram_tensor("qT", [KQ, NREF], SD, kind="ExternalInput")
    wq_d = nc.dram_tensor("wq", [KQ, 2 * KQ], SD, kind="ExternalInput")
    wkT_d = nc.dram_tensor("wkT", [KQ, 2 * KQ], SD, kind="ExternalInput")
    wo_d = nc.dram_tensor("wo", [LD, 2 * LD], VD, kind="ExternalInput")
    bq_d = nc.dram_tensor("bq2", [2, KQ], F32, kind="ExternalInput")
    out_d = nc.dram_tensor("out", [NREF, LD], F32, kind="ExternalOutput")

    inv_sqrt_kq = float(1.0 / np.sqrt(KQ))

    with tile.TileContext(nc) as tc:
        with tc.tile_pool(name="const", bufs=1) as cp, \
             tc.tile_pool(name="work", bufs=2) as sp, \
             tc.tile_pool(name="ps", bufs=2, space="PSUM") as pp:

            ones_col = cp.tile([128, 1], VD)
            nc.vector.memset(ones_col[:], 1.0)
            one11 = cp.tile([1, 1], F32)
            nc.vector.memset(one11[:], 1.0)

            kT = cp.tile([KQ, T], SD)
            nc.sync.dma_start(out=kT[:], in_=kT_d[:])
            xall = cp.tile([128, T], VD)  # x chunk c at cols [c*128,(c+1)*128)
            nc.sync.dma_start(out=xall[:], in_=x_d[:])
            qT_sb = cp.tile([KQ, NREF], SD)
            nc.scalar.dma_start(out=qT_sb[:], in_=qT_d[:])
            wq_sb = cp.tile([KQ, 2 * KQ], SD)
            nc.scalar.dma_start(out=wq_sb[:], in_=wq_d[:])
            wkT_sb = cp.tile([KQ, 2 * KQ], SD)
            nc.scalar.dma_start(out=wkT_sb[:], in_=wkT_d[:])
            wo_sb = cp.tile([LD, 2 * LD], VD)  # head h at cols [h*LD,(h+1)*LD)
            nc.scalar.dma_start(out=wo_sb[:], in_=wo_d[:])
            bq_sb = cp.tile([KQ, 2], F32)
            for h in range(2):
                nc.scalar.dma_start(out=bq_sb[:, h:h + 1], in_=bq_d[h, :, None])

            # ---- hq^T per local head, then m_h = Wk_h @ hq_h^T.
            # scores^T = k_in^T-chunks(stationary) @ m_h; the bk cross-term
            # is constant over keys and cancels exactly in the softmax.
            hqs, ms = [], []
            for h in range(2):
                qp = pp.tile([128, NREF], F32, tag="s1", bufs=4)
                nc.tensor.matmul(out=qp[:],
                                 lhsT=wq_sb[:, h * 128:(h + 1) * 128],
                                 rhs=qT_sb[:], start=True, stop=True)
                qs = sp.tile([128, NREF], SD, tag="hqs", bufs=2)
                nc.vector.tensor_scalar_add(out=qs[:], in0=qp[:],
                                            scalar1=bq_sb[:, h:h + 1])
                hqs.append(qs)
            for h in range(2):
                mp = pp.tile([128, NREF], F32, tag="s1", bufs=4)
                nc.tensor.matmul(out=mp[:],
                                 lhsT=wkT_sb[:, h * 128:(h + 1) * 128],
                                 rhs=hqs[h][:], start=True, stop=True)
                mb = sp.tile([128, NREF], SD, tag="ms", bufs=2)
                if h == 0:
                    nc.vector.tensor_copy(out=mb[:], in_=mp[:])
                else:
                    nc.scalar.copy(out=mb[:], in_=mp[:])
                ms.append(mb)

            # ---- scores^T chunks + exp (softmax numerator, unnormalized)
            # p~^T for head h, chunk c lives at pT_all[:, h*T + c*128 ...]
            pT_all = cp.tile([128, 2 * T], VD)
            for h in range(2):
                for cg in range(TCH // 4):
                    sc4 = pp.tile([128, 512], F32, tag="s1", bufs=4)
                    for j in range(4):
                        c = cg * 4 + j
                        nc.tensor.matmul(
                            out=sc4[:, j * 128:(j + 1) * 128],
                            lhsT=kT[:, c * 128:(c + 1) * 128],
                            rhs=ms[h][:], start=True, stop=True)
                    nc.scalar.activation(
                        out=pT_all[:, h * T + cg * 512: h * T + (cg + 1) * 512],
                        in_=sc4[:], func=AF.Exp, scale=inv_sqrt_kq)

            # ---- softmax denominators: Z_h[q] as a row via ones^T @ p~T,
            # transposed to a column by a second tiny matmul, then 1/Z.
            zrow = [pp.tile([1, NREF], F32, tag="s1", bufs=4, name=f"zr{h}")
                    for h in range(2)]
            for c in range(TCH):
                for h in range(2):
                    nc.tensor.matmul(
                        out=zrow[h][:], lhsT=ones_col[:],
                        rhs=pT_all[:, h * T + c * 128: h * T + (c + 1) * 128],
                        start=(c == 0), stop=(c == TCH - 1))

            # ---- value matmul: out_h^T[v, q] += x_c^T(stationary) @ p~T_c
            vo = [pp.tile([128, NREF], F32, tag="w2", bufs=2, name=f"vo{h}")
                  for h in range(2)]
            for c in range(TCH):
                for h in range(2):
                    nc.tensor.matmul(
                        out=vo[h][:],
                        lhsT=xall[:, c * 128:(c + 1) * 128],
                        rhs=pT_all[:, h * T + c * 128: h * T + (c + 1) * 128],
                        start=(c == 0), stop=(c == TCH - 1))

            rinv = []
            for h in range(2):
                zr_sb = sp.tile([1, NREF], F32, tag="zrs", bufs=2)
                nc.vector.tensor_copy(out=zr_sb[:], in_=zrow[h][:])
                zc_ps = pp.tile([NREF, 1], F32, tag="s1", bufs=4)
                nc.tensor.matmul(out=zc_ps[:], lhsT=zr_sb[:], rhs=one11[:],
                                 start=True, stop=True)
                zc_sb = sp.tile([NREF, 1], F32, tag="zcs", bufs=2)
                nc.vector.tensor_copy(out=zc_sb[:], in_=zc_ps[:])
                ri = sp.tile([NREF, 1], F32, tag="ri", bufs=2)
                nc.vector.reciprocal(out=ri[:], in_=zc_sb[:])
                rinv.append(ri)

            # ---- output projection per head, then normalize+combine
            fin = []
            for h in range(2):
                ot = sp.tile([128, NREF], VD, tag="ots", bufs=2)
                if h == 0:
                    nc.vector.tensor_copy(out=ot[:], in_=vo[h][:])
                else:
                    nc.scalar.copy(out=ot[:], in_=vo[h][:])
                fp = pp.tile([NREF, LD], F32, tag="s1", bufs=4, name=f"fin{h}")
                nc.tensor.matmul(out=fp[:], lhsT=ot[:],
                                 rhs=wo_sb[:, h * LD:(h + 1) * LD],
                                 start=True, stop=True)
                fin.append(fp)

            res0 = sp.tile([NREF, LD], F32, tag="res0", bufs=1)
            res1 = sp.tile([NREF, LD], F32, tag="res1", bufs=1)
            nc.vector.tensor_scalar_mul(out=res0[:], in0=fin[0][:],
                                        scalar1=rinv[0][:, :1])
            nc.vector.tensor_scalar_mul(out=res1[:], in0=fin[1][:],
                                        scalar1=rinv[1][:, :1])
            nc.vector.tensor_add(out=res0[:], in0=res0[:], in1=res1[:])
            nc.sync.dma_start(out=out_d[:], in_=res0[:])

    nc.compile()
    return nc


def _get_program(vd_name=None):
    vd_name = vd_name or VALUE_DTYPE
    if vd_name not in _CACHE:
        _CACHE[vd_name] = _build_program(vd_name)
    return _CACHE[vd_name]


def _host_prep(ts, ys0, ys1, emb0, emb1):
    """Full k_in^T (permuted) per batch and q_in^T."""
    div = np.exp(np.arange(0, DT, 2, dtype=np.float32)
                 * (-np.log(10.0) / DT)).astype(np.float32)  # (32,)
    ang = 48.0 * ts[:, :, None].astype(np.float32) * div[None, None, :]
    kT = np.empty((N, KQ, T), np.float32)
    kT[:, 0:32] = np.sin(ang).transpose(0, 2, 1)
    kT[:, 32:64] = np.cos(ang).transpose(0, 2, 1)
    kT[:, 64:96] = emb0[ys0].transpose(0, 2, 1)
    kT[:, 96:128] = emb1[ys1].transpose(0, 2, 1)

    ref = np.linspace(0.0, 1.0, NREF, dtype=np.float32)
    ang_r = 48.0 * ref[:, None] * div[None, :]  # (NREF, 32)
    qT = np.empty((KQ, NREF), np.float32)
    qT[0:32] = np.sin(ang_r).T
    qT[32:64] = np.cos(ang_r).T
    qT[64:96] = emb0[100][:, None]
    qT[96:128] = emb1[50][:, None]
    return kT, qT


def _make_in_maps(ts, ys0, ys1, x, emb0, emb1, Wq, bq, Wk, bk, Wo, vd_name):
    if vd_name == "f16":
        sd = vd = np.float16
    elif vd_name == "bf16":
        sd = vd = ml_dtypes.bfloat16
    else:  # hybrid
        sd, vd = ml_dtypes.bfloat16, np.float32
    bf = sd
    ts = np.asarray(ts, np.float32)
    x = np.asarray(x, np.float32)
    emb0 = np.asarray(emb0, np.float32)
    emb1 = np.asarray(emb1, np.float32)
    ys0 = np.asarray(ys0).astype(np.int64)
    ys1 = np.asarray(ys1).astype(np.int64)

    kT, qT = _host_prep(ts, ys0, ys1, emb0, emb1)
    # KQ permutation: (sin block | cos block | emb0 | emb1) -> reference order
    perm = np.concatenate([2 * np.arange(32), 2 * np.arange(32) + 1,
                           64 + np.arange(32), 96 + np.arange(32)])
    Wq_p = np.asarray(Wq, np.float32)[perm]
    Wk_p = np.asarray(Wk, np.float32)[perm]
    bq2 = np.asarray(bq, np.float32).reshape(H, KQ)
    bk2 = np.asarray(bk, np.float32).reshape(H, KQ)
    Wo = np.asarray(Wo, np.float32)
    # x rearranged: chunk c on cols [c*128,(c+1)*128), key t=c*128+p on part p
    xr = np.ascontiguousarray(
        x.reshape(N, TCH, 128, LD).transpose(0, 2, 1, 3).reshape(N, 128, T))

    in_maps = []
    for c in range(NCORES):
        b, hg = c // 2, c % 2
        # wo laid out (LD, 2*LD): local head h rows at cols [h*LD,(h+1)*LD)
        wo2 = np.ascontiguousarray(
            Wo[hg * 256:(hg + 1) * 256, :].reshape(2, LD, LD)
            .transpose(1, 0, 2).reshape(LD, 2 * LD))
        in_maps.append(dict(
            kT=kT[b].astype(bf),
            xr=xr[b].astype(vd),
            qT=qT.astype(bf),
            wq=np.ascontiguousarray(Wq_p[:, hg * 256:(hg + 1) * 256]).astype(bf),
            wkT=np.ascontiguousarray(
                Wk_p[:, hg * 256:(hg + 1) * 256].reshape(KQ, 2, KQ)
                .transpose(2, 1, 0).reshape(KQ, 2 * KQ)).astype(bf),
            wo=wo2.astype(vd),
            bq2=np.ascontiguousarray(bq2[2 * hg:2 * hg + 2]),
        ))
    return in_maps


def kernel(ts, ys0, ys1, x, emb0, emb1, Wq, bq, Wk, bk, Wo, bo):
    in_maps = _make_in_maps(ts, ys0, ys1, x, emb0, emb1, Wq, bq, Wk, bk, Wo,
                            VALUE_DTYPE)
    nc = _get_program()
    res = run_bass_kernel_spmd(nc, in_maps, list(range(NCORES)))
    bo = np.asarray(bo, np.float32)
    out = np.empty((N, NREF, LD), np.float32)
    for b in range(N):
        out[b] = (res.results[2 * b]["out"] + res.results[2 * b + 1]["out"]
                  + bo[None, :])
    return out
